# revision 26
# baseline (speedup 1.0000x reference)
"""Trainium2 Bass kernel for dynamic-conv1d attention-scale module.

Computes out = x + x * scale where
  scale[b,c,h,w] = sum_k attn[b,k,h,w] * w_sum[k,c]
  attn = softmax_k(logits/T),  logits[b,k,h,w] = fc2 @ relu(fc1 * qm)
  w_sum = weight.sum(axis=1)

Device strategy (8 NeuronCores, data-parallel over batch x H-halves):
  * quality_map >= 0 and fc1 is a bias-free 1x1 conv =>
    relu(fc1_w * q) == q * relu(fc1_w), so logits[k] = g[k]*q + b2[k]
    with g = fc2_w @ relu(fc1_w) (host-side weight-only folding).
  * softmax rows sum to 1 => 1 + scale = sum_k attn_k * (w_sum[k,c] + 1),
    so one tiny PE matmul per tile produces (1+scale) in PSUM and one
    vector multiply forms the output tile.
  * The matmul runs in float32r (full-rate fp32). Full fp32 accuracy is
    recovered with a 3-term compensated product folded into a single
    contract-dim-12 matmul: hi@w1_hi + lo@w1_hi + hi@w1_lo, where
    attn = hi + lo is split on-device at the f32r grid and w1 = w1_hi +
    w1_lo is split on host at the bf16 grid (bf16 is exactly
    representable in f32r).
  * Attention rows are transposed to pixel-major via a DRAM bounce and
    read back per 2048-pixel chunk so the first matmul doesn't wait for
    the full-row transfer under x-stream DMA contention.
  * x and y stream through HBM as bf16 (host converts both ways), which
    halves the dominant traffic vs fp32.  The rounding is multiplicative
    (err ~ 2^-9 * |out| per cast), so the cancellation-sensitive scale
    path keeps its full fp32 compensated precision and the max-rel
    metric is essentially unchanged.
Each core streams its 9.4 MB x-shard in [128 x 2048] bf16 tiles (512 KiB
DMAs), keeping the kernel at the HBM roofline (~20 MB of traffic/core).
"""

import sys

if "/opt/trn_rl_repo" not in sys.path:
    sys.path.insert(0, "/opt/trn_rl_repo")

import ml_dtypes
import numpy as np

import concourse.bacc as bacc
import concourse.mybir as mybir
from concourse.bass_utils import run_bass_kernel_spmd
from concourse.tile import TileContext

_B, _C, _H, _W = 4, 256, 192, 192
_K = 4
_TEMP = 34.0
_NCORES = 8
_HS = _H // 2            # 96 rows of H per shard
_N = _HS * _W            # 18432 pixels per core
_P = 128                 # SBUF partitions
_AP = 128                # partitions for attention pointwise math
_AF = _N // _AP          # 144 pixels per partition (576B DMA runs)
_CH = 2048               # pixels per main-loop tile (8 KB/partition)
_NT = _N // _CH          # 9 chunks
_MM = 512                # matmul moving free dim (one PSUM bank)
_DT = mybir.dt.float32
_DTR = mybir.dt.float32r
_BF = mybir.dt.bfloat16


def _build_nc():
    nc = bacc.Bacc()
    x_d = nc.dram_tensor("x", [_C, _N], _BF, kind="ExternalInput")
    qm_d = nc.dram_tensor("qm", [_AP, _AF], _DT, kind="ExternalInput")
    w_d = nc.dram_tensor("w", [3 * _K, _C], _DTR, kind="ExternalInput")
    g_d = nc.dram_tensor("g", [_AP, 2 * _K], _DT, kind="ExternalInput")
    y_d = nc.dram_tensor("y", [_C, _N], _BF, kind="ExternalOutput")
    # Plain per-partition dump of [hi | lo | hi] attention rows: the
    # transposing gather happens on the read side (chunked, pipelined)
    # so the prologue-critical write is 128 big contiguous descriptors.
    rows_s = nc.dram_tensor("rows_scratch", [_AP, 3 * _K * _AF], _DTR)

    KF = _K * _AF        # cols per hi/lo set in the [128, .] layout

    with TileContext(nc) as tc:
        with (
            tc.tile_pool(name="const", bufs=1) as cpool,
            tc.tile_pool(name="attn", bufs=1) as apool,
            tc.tile_pool(name="rowring", bufs=6) as rpool,
            tc.tile_pool(name="xin", bufs=12) as xpool,
            tc.tile_pool(name="sc", bufs=3) as spool,
            tc.tile_pool(name="yout", bufs=6) as ypool,
            tc.tile_pool(name="ps", bufs=2, space="PSUM") as pspool,
        ):
            # Dummy Exp on a zero tile: pulls the ACT LUT load off the
            # critical path (it otherwise fires only when the first real
            # exp dispatches, after the quality-map DMA lands).
            dum = apool.tile([1, 2], _DT)
            nc.vector.memzero(dum[:, :])
            nc.scalar.activation(
                out=dum[:, 1:2],
                in_=dum[:, 0:1],
                func=mybir.ActivationFunctionType.Exp,
            )
            # Small loads ride the sync queue AHEAD of the x stream so their
            # data + semaphores land with minimal latency.
            wt = cpool.tile([3 * _K, _C], _DTR)   # [w1_hi; w1_hi; w1_lo]
            gt = cpool.tile([_AP, 2 * _K], _DT)
            q = apool.tile([_AP, _AF], _DT)
            nc.sync.dma_start(out=q[:, :], in_=qm_d[:, :])
            nc.sync.dma_start(out=gt[:, :], in_=g_d[:, :])
            nc.sync.dma_start(out=wt[:, :], in_=w_d[:, :])

            # ---- attention pointwise in [128, 144] layout ----
            # logits on the DVE (4 cheap tensor_scalar ops), one wide Exp
            # on ACT: fewer serial dispatch gaps on the prologue chain.
            lg = apool.tile([_AP, KF], _DT)
            for k in range(_K):
                nc.vector.tensor_scalar(
                    out=lg[:, k * _AF : (k + 1) * _AF],
                    in0=q[:, :],
                    scalar1=gt[:, k : k + 1],
                    scalar2=gt[:, _K + k : _K + k + 1],
                    op0=mybir.AluOpType.mult,
                    op1=mybir.AluOpType.add,
                )
            e = apool.tile([_AP, KF], _DT)
            nc.scalar.activation(
                out=e[:, :],
                in_=lg[:, :],
                func=mybir.ActivationFunctionType.Exp,
            )
            d0 = apool.tile([_AP, _AF], _DT)
            d1 = apool.tile([_AP, _AF], _DT)
            nc.vector.tensor_add(
                out=d0[:, :], in0=e[:, 0:_AF], in1=e[:, _AF : 2 * _AF]
            )
            nc.vector.tensor_add(
                out=d1[:, :], in0=e[:, 2 * _AF : 3 * _AF], in1=e[:, 3 * _AF :]
            )
            nc.vector.tensor_add(out=d0[:, :], in0=d0[:, :], in1=d1[:, :])
            r = apool.tile([_AP, _AF], _DT)
            nc.vector.reciprocal_approx_accurate(
                out=r[:, :], in_=d0[:, :], scratch=d1[:, :]
            )
            # attn (full fp32) computed in place over e
            for k in range(_K):
                nc.vector.tensor_mul(
                    out=e[:, k * _AF : (k + 1) * _AF],
                    in0=e[:, k * _AF : (k + 1) * _AF],
                    in1=r[:, :],
                )
            ahl = apool.tile([_AP, 2 * KF], _DTR)  # [hi | lo]
            nc.vector.tensor_copy(out=ahl[:, 0:KF], in_=e[:, :])
            nc.vector.tensor_sub(
                out=ahl[:, KF : 2 * KF],
                in0=e[:, :],
                in1=ahl[:, 0:KF].bitcast(_DT),
            )
            # Straight dumps (no rearrange): [hi | lo] then the hi repeat,
            # split into 16-partition blocks so early chunks' gathers gate
            # only on tiny early dumps (the x prefetch saturates HBM here,
            # so a monolithic dump would stall the first matmul ~10us).
            for pb in range(0, _AP, 16):
                nc.scalar.dma_start(
                    out=rows_s[pb : pb + 16, 0 : 2 * KF],
                    in_=ahl[pb : pb + 16, :],
                )
                nc.scalar.dma_start(
                    out=rows_s[pb : pb + 16, 2 * KF : 3 * KF],
                    in_=ahl[pb : pb + 16, 0:KF],
                )
            # Pixel-major [3K, N] view of the dump for the chunked gathers.
            rows_v = rows_s.rearrange("p (s f) -> s p f", s=3 * _K)

            # ---- main stream: out = x * (1 + scale) ----
            # The PSUM->SBUF bf16 copy runs on the scalar engine so the
            # final multiply is bf16 x bf16 on the DVE (2x packed mode).
            # y DMAs are issued on the scalar ring lagged by 2 tiles so
            # their sequencer-side waits (on the DVE multiply) are always
            # already satisfied and never stall the copy stream.
            pending = []
            for t in range(_NT):
                nsl = slice(t * _CH, (t + 1) * _CH)
                # Chunked pixel-major rows gather from the dump: up to 3
                # rectangular regions (chunk bounds don't align to the
                # per-partition _AF runs).
                rt = rpool.tile([3 * _K, _CH], _DTR)
                a, b = t * _CH, (t + 1) * _CH
                p_lo, f_lo = divmod(a, _AF)
                p_hi, f_hi = divmod(b, _AF)
                off = 0
                if f_lo:
                    ln = _AF - f_lo
                    nc.gpsimd.dma_start(
                        out=rt[:, 0:ln].rearrange("s (p f) -> s p f", p=1),
                        in_=rows_v[:, p_lo : p_lo + 1, f_lo:_AF],
                    )
                    off = ln
                    p_lo += 1
                npm = p_hi - p_lo
                nc.gpsimd.dma_start(
                    out=rt[:, off : off + npm * _AF].rearrange(
                        "s (p f) -> s p f", p=npm
                    ),
                    in_=rows_v[:, p_lo:p_hi, :],
                )
                if f_hi:
                    nc.gpsimd.dma_start(
                        out=rt[:, _CH - f_hi : _CH].rearrange(
                            "s (p f) -> s p f", p=1
                        ),
                        in_=rows_v[:, p_hi : p_hi + 1, 0:f_hi],
                    )
                for ch in range(_C // _P):
                    lhsT = wt[:, ch * _P : (ch + 1) * _P]
                    xt = xpool.tile([_P, _CH], _BF)
                    nc.sync.dma_start(
                        out=xt[:, :], in_=x_d[ch * _P : (ch + 1) * _P, nsl]
                    )
                    ps = pspool.tile([_P, _CH], _DT)
                    for j in range(_CH // _MM):
                        nc.tensor.matmul(
                            ps[:, j * _MM : (j + 1) * _MM],
                            lhsT,
                            rt[:, j * _MM : (j + 1) * _MM],
                            start=True,
                            stop=True,
                        )
                    ot = ypool.tile([_P, _CH], _BF)
                    if t * 2 + ch < 14:
                        # split path: ACT converts PSUM->bf16, DVE runs the
                        # packed 2x bf16 multiply
                        st = spool.tile([_P, _CH], _BF)
                        nc.scalar.activation(
                            out=st[:, :],
                            in_=ps[:, :],
                            func=mybir.ActivationFunctionType.Copy,
                        )
                        nc.vector.tensor_mul(
                            out=ot[:, :], in0=xt[:, :], in1=st[:, :]
                        )
                    else:
                        # direct path (tail): one DVE op, shorter drain chain
                        nc.vector.tensor_mul(
                            out=ot[:, :], in0=xt[:, :], in1=ps[:, :]
                        )
                    pending.append((ot, ch * _P, nsl))
                    if len(pending) > 2:
                        po, pc, pn = pending.pop(0)
                        nc.scalar.dma_start(
                            out=y_d[pc : pc + _P, pn], in_=po[:, :]
                        )
            for po, pc, pn in pending:
                nc.scalar.dma_start(out=y_d[pc : pc + _P, pn], in_=po[:, :])
    nc.compile()
    return nc


def _prepare_in_maps(x, quality_map, fc1_w, fc2_w, fc2_b, weight):
    x = np.asarray(x, dtype=np.float32)
    qm = np.asarray(quality_map, dtype=np.float32)
    fc1 = np.asarray(fc1_w, dtype=np.float32)
    fc2 = np.asarray(fc2_w, dtype=np.float32)
    b2 = np.asarray(fc2_b, dtype=np.float32)
    w = np.asarray(weight, dtype=np.float32)

    # Weight-only folding (host): g = fc2 @ relu(fc1); w1 = w_sum + 1,
    # split at the bf16 grid: w1 = w1_hi + w1_lo (w1_hi exact in f32r).
    g = (fc2 @ np.maximum(fc1[:, 0], 0.0)).astype(np.float32)        # [K]
    w1 = (w.sum(axis=1) + 1.0).astype(np.float32)                    # [K, C]
    w1_hi = w1.astype(ml_dtypes.bfloat16).astype(np.float32)
    w1_lo = (w1 - w1_hi).astype(np.float32)
    # Pairs with rows3 = [hi; lo; hi]:
    wstack = np.concatenate([w1_hi, w1_hi, w1_lo], axis=0)           # [12, C]
    gb = np.concatenate([g / _TEMP, b2 / _TEMP]).astype(np.float32)  # [2K]
    gb_rep = np.ascontiguousarray(np.broadcast_to(gb, (_AP, 2 * _K)))

    in_maps = []
    xb = x.astype(ml_dtypes.bfloat16)
    for core in range(_NCORES):
        b, half = divmod(core, 2)
        h0 = half * _HS
        xs = np.ascontiguousarray(xb[b, :, h0 : h0 + _HS, :]).reshape(_C, _N)
        qs = np.ascontiguousarray(qm[b, 0, h0 : h0 + _HS, :]).reshape(_AP, _AF)
        in_maps.append({"x": xs, "qm": qs, "w": wstack, "g": gb_rep})
    return in_maps


def _run(in_maps, **kwargs):
    nc = _build_nc()
    return run_bass_kernel_spmd(nc, in_maps, core_ids=list(range(_NCORES)), **kwargs)


def kernel(x, quality_map, fc1_w, fc2_w, fc2_b, weight):
    in_maps = _prepare_in_maps(x, quality_map, fc1_w, fc2_w, fc2_b, weight)
    res = _run(in_maps)
    out = np.empty((_B, _C, _H, _W), dtype=np.float32)
    for core in range(_NCORES):
        b, half = divmod(core, 2)
        h0 = half * _HS
        ys = np.asarray(res.results[core]["y"]).astype(np.float32)
        out[b, :, h0 : h0 + _HS, :] = ys.reshape(_C, _HS, _W)
    return out



# revision 27
# speedup vs baseline: 1.0016x; 1.0016x over previous
"""Trainium2 Bass kernel for dynamic-conv1d attention-scale module.

Computes out = x + x * scale where
  scale[b,c,h,w] = sum_k attn[b,k,h,w] * w_sum[k,c]
  attn = softmax_k(logits/T),  logits[b,k,h,w] = fc2 @ relu(fc1 * qm)
  w_sum = weight.sum(axis=1)

Device strategy (8 NeuronCores, data-parallel over batch x H-halves):
  * quality_map >= 0 and fc1 is a bias-free 1x1 conv =>
    relu(fc1_w * q) == q * relu(fc1_w), so logits[k] = g[k]*q + b2[k]
    with g = fc2_w @ relu(fc1_w) (host-side weight-only folding).
  * softmax rows sum to 1 => 1 + scale = sum_k attn_k * (w_sum[k,c] + 1),
    so one tiny PE matmul per tile produces (1+scale) in PSUM and one
    vector multiply forms the output tile.
  * The matmul runs in float32r (full-rate fp32). Full fp32 accuracy is
    recovered with a 3-term compensated product folded into a single
    contract-dim-12 matmul: hi@w1_hi + lo@w1_hi + hi@w1_lo, where
    attn = hi + lo is split on-device at the f32r grid and w1 = w1_hi +
    w1_lo is split on host at the bf16 grid (bf16 is exactly
    representable in f32r).
  * Attention rows are transposed to pixel-major via a DRAM bounce and
    read back per 2048-pixel chunk so the first matmul doesn't wait for
    the full-row transfer under x-stream DMA contention.
  * x and y stream through HBM as bf16 (host converts both ways), which
    halves the dominant traffic vs fp32.  The rounding is multiplicative
    (err ~ 2^-9 * |out| per cast), so the cancellation-sensitive scale
    path keeps its full fp32 compensated precision and the max-rel
    metric is essentially unchanged.
Each core streams its 9.4 MB x-shard in [128 x 2048] bf16 tiles (512 KiB
DMAs), keeping the kernel at the HBM roofline (~20 MB of traffic/core).
"""

import sys

if "/opt/trn_rl_repo" not in sys.path:
    sys.path.insert(0, "/opt/trn_rl_repo")

import ml_dtypes
import numpy as np

import concourse.bacc as bacc
import concourse.mybir as mybir
from concourse.bass_utils import run_bass_kernel_spmd
from concourse.tile import TileContext

_B, _C, _H, _W = 4, 256, 192, 192
_K = 4
_TEMP = 34.0
_NCORES = 8
_HS = _H // 2            # 96 rows of H per shard
_N = _HS * _W            # 18432 pixels per core
_P = 128                 # SBUF partitions
_AP = 128                # partitions for attention pointwise math
_AF = _N // _AP          # 144 pixels per partition (576B DMA runs)
_CH = 2048               # pixels per main-loop tile (8 KB/partition)
_NT = _N // _CH          # 9 chunks
_MM = 512                # matmul moving free dim (one PSUM bank)
_DT = mybir.dt.float32
_DTR = mybir.dt.float32r
_BF = mybir.dt.bfloat16


def _build_nc():
    nc = bacc.Bacc()
    x_d = nc.dram_tensor("x", [_C, _N], _BF, kind="ExternalInput")
    qm_d = nc.dram_tensor("qm", [_AP, _AF], _DT, kind="ExternalInput")
    w_d = nc.dram_tensor("w", [3 * _K, _C], _DTR, kind="ExternalInput")
    g_d = nc.dram_tensor("g", [_AP, 2 * _K], _DT, kind="ExternalInput")
    y_d = nc.dram_tensor("y", [_C, _N], _BF, kind="ExternalOutput")
    # Plain per-partition dump of [hi | lo | hi] attention rows: the
    # transposing gather happens on the read side (chunked, pipelined)
    # so the prologue-critical write is 128 big contiguous descriptors.
    rows_s = nc.dram_tensor("rows_scratch", [_AP, 3 * _K * _AF], _DTR)

    KF = _K * _AF        # cols per hi/lo set in the [128, .] layout

    with TileContext(nc) as tc:
        with (
            tc.tile_pool(name="const", bufs=1) as cpool,
            tc.tile_pool(name="attn", bufs=1) as apool,
            tc.tile_pool(name="rowring", bufs=6) as rpool,
            tc.tile_pool(name="xin", bufs=12) as xpool,
            tc.tile_pool(name="sc", bufs=3) as spool,
            tc.tile_pool(name="yout", bufs=6) as ypool,
            tc.tile_pool(name="ps", bufs=2, space="PSUM") as pspool,
        ):
            # Dummy Exp on a zero tile: pulls the ACT LUT load off the
            # critical path (it otherwise fires only when the first real
            # exp dispatches, after the quality-map DMA lands).
            dum = apool.tile([1, 2], _DT)
            nc.vector.memzero(dum[:, :])
            nc.scalar.activation(
                out=dum[:, 1:2],
                in_=dum[:, 0:1],
                func=mybir.ActivationFunctionType.Exp,
            )
            # Small loads ride the sync queue AHEAD of the x stream so their
            # data + semaphores land with minimal latency.
            wt = cpool.tile([3 * _K, _C], _DTR)   # [w1_hi; w1_hi; w1_lo]
            gt = cpool.tile([_AP, 2 * _K], _DT)
            q = apool.tile([_AP, _AF], _DT)
            nc.scalar.dma_start(out=q[:, :], in_=qm_d[:, :])
            nc.scalar.dma_start(out=gt[:, :], in_=g_d[:, :])
            nc.sync.dma_start(out=wt[:, :], in_=w_d[:, :])

            # ---- attention pointwise in [128, 144] layout ----
            # logits on the DVE (4 cheap tensor_scalar ops), one wide Exp
            # on ACT: fewer serial dispatch gaps on the prologue chain.
            lg = apool.tile([_AP, KF], _DT)
            for k in range(_K):
                nc.vector.tensor_scalar(
                    out=lg[:, k * _AF : (k + 1) * _AF],
                    in0=q[:, :],
                    scalar1=gt[:, k : k + 1],
                    scalar2=gt[:, _K + k : _K + k + 1],
                    op0=mybir.AluOpType.mult,
                    op1=mybir.AluOpType.add,
                )
            e = apool.tile([_AP, KF], _DT)
            nc.scalar.activation(
                out=e[:, :],
                in_=lg[:, :],
                func=mybir.ActivationFunctionType.Exp,
            )
            d0 = apool.tile([_AP, _AF], _DT)
            d1 = apool.tile([_AP, _AF], _DT)
            nc.vector.tensor_add(
                out=d0[:, :], in0=e[:, 0:_AF], in1=e[:, _AF : 2 * _AF]
            )
            nc.vector.tensor_add(
                out=d1[:, :], in0=e[:, 2 * _AF : 3 * _AF], in1=e[:, 3 * _AF :]
            )
            nc.vector.tensor_add(out=d0[:, :], in0=d0[:, :], in1=d1[:, :])
            r = apool.tile([_AP, _AF], _DT)
            nc.vector.reciprocal_approx_accurate(
                out=r[:, :], in_=d0[:, :], scratch=d1[:, :]
            )
            # attn (full fp32) computed in place over e
            for k in range(_K):
                nc.vector.tensor_mul(
                    out=e[:, k * _AF : (k + 1) * _AF],
                    in0=e[:, k * _AF : (k + 1) * _AF],
                    in1=r[:, :],
                )
            ahl = apool.tile([_AP, 2 * KF], _DTR)  # [hi | lo]
            nc.vector.tensor_copy(out=ahl[:, 0:KF], in_=e[:, :])
            nc.vector.tensor_sub(
                out=ahl[:, KF : 2 * KF],
                in0=e[:, :],
                in1=ahl[:, 0:KF].bitcast(_DT),
            )
            # Straight dumps (no rearrange): [hi | lo] then the hi repeat,
            # split into 16-partition blocks so early chunks' gathers gate
            # only on tiny early dumps (the x prefetch saturates HBM here,
            # so a monolithic dump would stall the first matmul ~10us).
            for pb in range(0, _AP, 16):
                nc.scalar.dma_start(
                    out=rows_s[pb : pb + 16, 0 : 2 * KF],
                    in_=ahl[pb : pb + 16, :],
                )
                nc.scalar.dma_start(
                    out=rows_s[pb : pb + 16, 2 * KF : 3 * KF],
                    in_=ahl[pb : pb + 16, 0:KF],
                )
            # Pixel-major [3K, N] view of the dump for the chunked gathers.
            rows_v = rows_s.rearrange("p (s f) -> s p f", s=3 * _K)

            # ---- main stream: out = x * (1 + scale) ----
            # The PSUM->SBUF bf16 copy runs on the scalar engine so the
            # final multiply is bf16 x bf16 on the DVE (2x packed mode).
            # y DMAs are issued on the scalar ring lagged by 2 tiles so
            # their sequencer-side waits (on the DVE multiply) are always
            # already satisfied and never stall the copy stream.
            pending = []
            for t in range(_NT):
                nsl = slice(t * _CH, (t + 1) * _CH)
                # Chunked pixel-major rows gather from the dump: up to 3
                # rectangular regions (chunk bounds don't align to the
                # per-partition _AF runs).
                rt = rpool.tile([3 * _K, _CH], _DTR)
                a, b = t * _CH, (t + 1) * _CH
                p_lo, f_lo = divmod(a, _AF)
                p_hi, f_hi = divmod(b, _AF)
                off = 0
                if f_lo:
                    ln = _AF - f_lo
                    nc.gpsimd.dma_start(
                        out=rt[:, 0:ln].rearrange("s (p f) -> s p f", p=1),
                        in_=rows_v[:, p_lo : p_lo + 1, f_lo:_AF],
                    )
                    off = ln
                    p_lo += 1
                npm = p_hi - p_lo
                nc.gpsimd.dma_start(
                    out=rt[:, off : off + npm * _AF].rearrange(
                        "s (p f) -> s p f", p=npm
                    ),
                    in_=rows_v[:, p_lo:p_hi, :],
                )
                if f_hi:
                    nc.gpsimd.dma_start(
                        out=rt[:, _CH - f_hi : _CH].rearrange(
                            "s (p f) -> s p f", p=1
                        ),
                        in_=rows_v[:, p_hi : p_hi + 1, 0:f_hi],
                    )
                for ch in range(_C // _P):
                    lhsT = wt[:, ch * _P : (ch + 1) * _P]
                    xt = xpool.tile([_P, _CH], _BF)
                    nc.sync.dma_start(
                        out=xt[:, :], in_=x_d[ch * _P : (ch + 1) * _P, nsl]
                    )
                    ps = pspool.tile([_P, _CH], _DT)
                    for j in range(_CH // _MM):
                        nc.tensor.matmul(
                            ps[:, j * _MM : (j + 1) * _MM],
                            lhsT,
                            rt[:, j * _MM : (j + 1) * _MM],
                            start=True,
                            stop=True,
                        )
                    ot = ypool.tile([_P, _CH], _BF)
                    if t * 2 + ch < 14:
                        # split path: ACT converts PSUM->bf16, DVE runs the
                        # packed 2x bf16 multiply
                        st = spool.tile([_P, _CH], _BF)
                        nc.scalar.activation(
                            out=st[:, :],
                            in_=ps[:, :],
                            func=mybir.ActivationFunctionType.Copy,
                        )
                        nc.vector.tensor_mul(
                            out=ot[:, :], in0=xt[:, :], in1=st[:, :]
                        )
                    else:
                        # direct path (tail): one DVE op, shorter drain chain
                        nc.vector.tensor_mul(
                            out=ot[:, :], in0=xt[:, :], in1=ps[:, :]
                        )
                    pending.append((ot, ch * _P, nsl))
                    if len(pending) > 2:
                        po, pc, pn = pending.pop(0)
                        nc.scalar.dma_start(
                            out=y_d[pc : pc + _P, pn], in_=po[:, :]
                        )
            for po, pc, pn in pending:
                nc.scalar.dma_start(out=y_d[pc : pc + _P, pn], in_=po[:, :])
    nc.compile()
    return nc


def _prepare_in_maps(x, quality_map, fc1_w, fc2_w, fc2_b, weight):
    x = np.asarray(x, dtype=np.float32)
    qm = np.asarray(quality_map, dtype=np.float32)
    fc1 = np.asarray(fc1_w, dtype=np.float32)
    fc2 = np.asarray(fc2_w, dtype=np.float32)
    b2 = np.asarray(fc2_b, dtype=np.float32)
    w = np.asarray(weight, dtype=np.float32)

    # Weight-only folding (host): g = fc2 @ relu(fc1); w1 = w_sum + 1,
    # split at the bf16 grid: w1 = w1_hi + w1_lo (w1_hi exact in f32r).
    g = (fc2 @ np.maximum(fc1[:, 0], 0.0)).astype(np.float32)        # [K]
    w1 = (w.sum(axis=1) + 1.0).astype(np.float32)                    # [K, C]
    w1_hi = w1.astype(ml_dtypes.bfloat16).astype(np.float32)
    w1_lo = (w1 - w1_hi).astype(np.float32)
    # Pairs with rows3 = [hi; lo; hi]:
    wstack = np.concatenate([w1_hi, w1_hi, w1_lo], axis=0)           # [12, C]
    gb = np.concatenate([g / _TEMP, b2 / _TEMP]).astype(np.float32)  # [2K]
    gb_rep = np.ascontiguousarray(np.broadcast_to(gb, (_AP, 2 * _K)))

    in_maps = []
    xb = x.astype(ml_dtypes.bfloat16)
    for core in range(_NCORES):
        b, half = divmod(core, 2)
        h0 = half * _HS
        xs = np.ascontiguousarray(xb[b, :, h0 : h0 + _HS, :]).reshape(_C, _N)
        qs = np.ascontiguousarray(qm[b, 0, h0 : h0 + _HS, :]).reshape(_AP, _AF)
        in_maps.append({"x": xs, "qm": qs, "w": wstack, "g": gb_rep})
    return in_maps


def _run(in_maps, **kwargs):
    nc = _build_nc()
    return run_bass_kernel_spmd(nc, in_maps, core_ids=list(range(_NCORES)), **kwargs)


def kernel(x, quality_map, fc1_w, fc2_w, fc2_b, weight):
    in_maps = _prepare_in_maps(x, quality_map, fc1_w, fc2_w, fc2_b, weight)
    res = _run(in_maps)
    out = np.empty((_B, _C, _H, _W), dtype=np.float32)
    for core in range(_NCORES):
        b, half = divmod(core, 2)
        h0 = half * _HS
        ys = np.asarray(res.results[core]["y"]).astype(np.float32)
        out[b, :, h0 : h0 + _HS, :] = ys.reshape(_C, _HS, _W)
    return out

